# revision 55
# baseline (speedup 1.0000x reference)
"""Trainium2 Bass kernel for CLIP attention pooling.

Reference computation (N=4096, D=1024, fp32):
    q = x @ Wq.T + bq
    k = x @ Wk.T + bk
    attn = softmax(q @ k.T, axis=-1)
    out = attn @ x

Math notes:
  * scores = q @ k.T = q @ Wk @ x.T + (q.bk) 1^T. The (q.bk) term is
    constant along the softmax axis, so bk never needs to be computed.
  * q @ Wk = x @ (Wq.T @ Wk) + bq @ Wk: both projections fold into one
    matrix M = Wq.T @ Wk and a row c = bq @ Wk (host-precomputed).
  * Per core (512 query rows):
        tT = M^T . xs^T + c          [D, 512]   (transposed layout)
        S  = t . x^T                 [512, 4096]
        P  = softmax(S)              (online, running-max)
        out = P @ x                  [512, 1024]
  * The whole score path runs in fp16 (10-bit mantissa ~ fp32r's 11);
    scores accumulate in fp32 PSUM. E is bf16, out is fp32. fp8
    (DoubleRow 2x) was evaluated and fails the accuracy budget: e4m3
    quantization of P/x alone costs ~3e-2 L2 on the output.

Schedule (PE busy ~131us of ~149us exec; B and C run at the 216ns/
512-cycle matmul roofline with <100ns/group of slack):
  * phase A: M chunks stream on the sync HWDGE ring, xs chunks on the
    scalar ring (independent trigger FIFOs; the first-needed pieces
    lead both rings, xs' second e0 half rides sync so round e0 is not
    gated on the slow scalar trigger queue). 42 tiny matmuls on a
    memset tile warm the PE p-state while the first chunks land - the
    DVFS ramp is activity-driven (half clock for ~5.5us) and the early
    DMA only sustains ~100-250GB/s, so real work cannot start before
    ~11.5us anyway and the warmups exactly cover ramp + DMA latency.
    e-outer over 8 PSUM banks for chunks e0-e4 (chasing the streams),
    then each bank finishes e5-e7 d-by-d so the 8 stops stagger
    ~0.65us apart; each bank's PSUM->SBUF copy (bias c folded in,
    full-width, engines alternating by d - R-range splits of one
    d-slice serialize across engines) chases its stop, and phase B's
    first group (which inherits ALL copies' deps via accumulation-
    group dep merging) starts ~0.8us after A's last matmul.
  * key-chunk rotation: each core processes key chunks in rotated order
    [c, c+1, ...]; its own query slice xTs IS rotated chunk 0 and is
    already in SBUF when phase B starts (zero DMA wait). xTb and xb are
    host-rotated to match; output rows are queries, so unaffected.
  * phase B: online softmax. Per (i, chunk): 8 accumulating matmuls
    into a PSUM bank, DVE keeps a running negated max straight out of
    PSUM, ACT applies exp(PSUM - runmax) into bf16 E with accum_out
    collecting per-chunk partial sums. No S buffer exists. Chunks 6-7
    run i0/i1-first so their finalize chain (c_k = exp(m_k - m_final),
    Z = sum z_k c_k, g_k = c_k/Z -> per-chunk diag(g) tiles) completes
    ~2 PE groups before B's last matmul; i3's final group sits on a
    dedicated PSUM bank (via pad tiles) so its exp can be deferred into
    phase C's queues without gating anything.
  * phase C: one pass per i-tile (out rows drain 3x earlier), emitted
    as a single continuous software pipeline; the oacc matmul groups
    lag LOOKP=3 behind and cross pass boundaries (accumulator banks
    alternate by pass parity), so no per-pass barrier exists.
  * E transposes: pass 0 transposes on the PE (E_tile.T @ diag(g)
    matmuls, 4 jt per pst bank + one PSUM->SBUF et copy per group -
    its g_k are only ready ~2 PE groups before B ends). Passes 1-3
    pay ZERO PE transpose cycles: their E tiles are rescaled in place
    (E *= g_k per chunk, DVE/ACT tensor-scalar, popped one pass
    ahead) and transposed by the DMA xbar (dma_start_transpose,
    16x128 hw tiles, ~3.6us per 1MB i-tile on the sync ring) into
    [key%128, key//128, q] etq buffers - exactly the lhsT tiling the
    accumulation matmuls need, ~3us before each pass consumes them.
    This cuts 24 of 32 transpose groups from the PE (~5us). Deferred
    phase-B finalize work pops ahead of the pass-0 transposes that
    read its diag tiles; drain pops wait for slot 3 of each pass.
  * x (bf16) is fully resident in SBUF: host-pretransposed to
    [128, 32, 1024], loaded in 4 quarters on the sync ring BEHIND the
    phase-B xtj stream (ring-FIFO order = zero bandwidth contention
    with the stream B chases; a third-ring/gpsimd variant measurably
    starved xtj and cost 2-5us of B). It aliases the released weight
    pools and still lands ~10us before phase C's first pass ends.
  * outputs: each 128-row block is copied from two PSUM banks into one
    [128, 1024] tile (DVE + ACT in parallel) and leaves via a single
    full-width DMA, rings alternating, hidden under the next pass; the
    final block leaves as ROW-halves on both rings with 4KB-row
    descriptors (2KB-row column-split DMAs only sustain ~100GB/s per
    queue).
"""

import os
from contextlib import ExitStack

import numpy as np
import ml_dtypes

import concourse.bass as bass
import concourse.mybir as mybir
import concourse.tile as tile
from concourse import bacc
from concourse.bass_utils import run_bass_kernel_spmd
from concourse.masks import make_identity

N, D = 4096, 1024
NCORES = 8
R = N // NCORES  # 512 query rows per core
PT = 128  # partition tile
EC = D // PT  # 8 contraction chunks of the model dim
IT = R // PT  # 4 query tiles per core
JC = N // 512  # 8 key chunks of 512
JT = N // PT  # 32 key tiles of 128

F32 = mybir.dt.float32
F32R = mybir.dt.float32r
F16 = mybir.dt.float16
BF16 = mybir.dt.bfloat16
AX = mybir.AxisListType
AF = mybir.ActivationFunctionType
ALU = mybir.AluOpType

PASSES = ((0, 1), (2,), (3,))


def _emit(nc: bass.Bass, tc: tile.TileContext, aps: dict):
    xTb, xTs, mw, cw, xb, out = (
        aps["xTb"], aps["xTs"], aps["mw"], aps["cw"],
        aps["xb"], aps["out"],
    )

    with ExitStack() as big:
        persist = big.enter_context(tc.tile_pool(name="persist", bufs=1))

        # warmup source: a plain memset tile (~100ns on gpsimd) so the PE
        # p-state ramp can start without waiting for the iota-based identity.
        wsrc = persist.tile([PT, PT], BF16)
        nc.gpsimd.memset(wsrc, 0.0)
        ident = persist.tile([PT, PT], BF16)
        make_identity(nc, ident)
        c_sb = persist.tile([PT, EC], F32)
        fence_sb = persist.tile([PT, 8], F16)

        tT_sb = persist.tile([PT, EC, R], F16)
        E_bf = [persist.tile([PT, N], BF16, name=f"E{i}") for i in range(IT)]
        nmk = [persist.tile([PT, JC], F32, name=f"nmk{i}") for i in range(IT)]
        tmx = [persist.tile([PT, JC], F32, name=f"tmx{i}") for i in range(IT)]
        zpart = [persist.tile([PT, JC], F32, name=f"zp{i}") for i in range(IT)]
        ck = [persist.tile([PT, JC], F32, name=f"ck{i}") for i in range(IT)]
        gk = [persist.tile([PT, JC], F32, name=f"gk{i}") for i in range(IT)]
        zsum = [persist.tile([PT, 1], F32, name=f"z{i}") for i in range(IT)]
        rz = [persist.tile([PT, 1], F32, name=f"rz{i}") for i in range(IT)]
        diag = persist.tile([PT, IT, JC, PT], BF16)
        # hoisted pass-0 et tiles (filled in B's tail; persist avoids any
        # pool-ordering constraints with xspool's mid-B close)
        etp = [persist.tile([PT, 512], BF16, name=f"etp{i}") for i in range(2)]

        # opened before wpool so its addresses never overlap the weights;
        # the early stream triggers can then issue during phase A.
        xtpool = big.enter_context(tc.tile_pool(name="xtpool", bufs=4))
        xtjs = {}
        for j in range(1, JC):
            xtjs[j] = xtpool.tile([PT, EC, 512], F16, tag="xtj", name="xtj")

        # xs in its own pool (opened after xtpool so pools unwind LIFO):
        # stream position 0 of phase B reads it directly - each core's query
        # slice IS its own key chunk - so it is only released after that;
        # the xb buffer then aliases it + wpool.
        xspool_cm = tc.tile_pool(name="xspool", bufs=1)
        xspool = xspool_cm.__enter__()
        xts_sb = xspool.tile([PT, EC, R], F16)

        # ---- Phase A: tT = M^T.xs^T + c  (transposed layout)
        with ExitStack() as pha:
            wpool = pha.enter_context(tc.tile_pool(name="wpool", bufs=1))
            apsum = pha.enter_context(tc.tile_pool(name="apsum", bufs=1, space="PSUM"))

            m_sb = wpool.tile([PT, EC, D], F16)

            m_r = mw.rearrange("(t p) d -> p t d", p=PT)
            xTs_r = xTs.rearrange("(t p) i -> p t i", p=PT)
            # M rides the sync HWDGE ring, xs + bias the scalar ring: the
            # trigger FIFOs are independent and the SDMA engines round-robin
            # between them. Chunk 0 of the phase-B stream is slotted in
            # before the last two M chunks: phase A's compute tail covers it.
            # First-needed pieces lead both rings: round e0 runs in R-halves,
            # so the very first matmuls need only m0a + the first 64KB of xs.
            # The e0h1 slice rides the sync ring (scalar's queue would issue
            # its trigger too late); c is tiny and not needed until A's tail,
            # so its trigger goes after the whole xs stream.
            nc.sync.dma_start(m_sb[:, 0, 0:PT], m_r[:, 0, 0:PT])
            nc.scalar.dma_start(xts_sb[:, 0, 0:256], xTs_r[:, 0, 0:256])
            nc.sync.dma_start(xts_sb[:, 0, 256:512], xTs_r[:, 0, 256:512])
            nc.sync.dma_start(m_sb[:, 0, PT:D], m_r[:, 0, PT:D])
            nc.scalar.dma_start(xts_sb[:, 1, :], xTs_r[:, 1, :])
            for e in range(1, EC):
                nc.sync.dma_start(m_sb[:, e, 0:512], m_r[:, e, 0:512])
                nc.sync.dma_start(m_sb[:, e, 512:D], m_r[:, e, 512:D])
            for e in range(2, EC):
                nc.scalar.dma_start(xts_sb[:, e, :], xTs_r[:, e, :])
            nc.scalar.dma_start(c_sb, cw)

            tps = [
                apsum.tile([PT, R], F32, tag=f"tp{d}", name=f"tp{d}")
                for d in range(EC)
            ]
            # PE warm-up only while the first DMA chunks are in flight
            # (~1.3us): the real phase-A matmuls then continue the p-state
            # ramp doing useful work at the reduced clock (the e-loop chases
            # the M/xs streams and stays stall-free even at half clock).
            # Results are clobbered by the first start=True matmul.
            for _ in range(42):
                nc.tensor.matmul(
                    tps[0][:, 0:PT], wsrc, wsrc, start=True, stop=True
                )
            # e-outer for chunks 0-4 (chasing the M/xs streams), then each
            # bank finishes its last 3 chunks d-by-d so the 8 stops stagger
            # ~0.65us apart instead of bunching in A's last round. Each
            # bank's PSUM->SBUF copy (bias c folded in) is issued right at
            # its stop, split into DVE+ACT halves running in parallel: the
            # copies fully chase the stops and B's first group (which
            # inherits ALL copies' deps via group-level merging) starts
            # ~0.4us after A's last matmul instead of ~1.3us.
            ESPLIT = 5
            for e in range(ESPLIT):
                for d in range(EC):
                    nc.tensor.matmul(
                        tps[d],
                        m_sb[:, e, d * PT : (d + 1) * PT],
                        xts_sb[:, e, :],
                        start=(e == 0),
                        stop=False,
                    )
            for d in range(EC):
                for e in range(ESPLIT, EC):
                    nc.tensor.matmul(
                        tps[d],
                        m_sb[:, e, d * PT : (d + 1) * PT],
                        xts_sb[:, e, :],
                        start=False,
                        stop=(e == EC - 1),
                    )
                # full-width copy, engines alternating by d: half-splits of
                # one d-slice get serialized DVE->ACT by the dep tracker.
                if d % 2 == 0:
                    nc.vector.tensor_scalar_add(
                        tT_sb[:, d, :], tps[d], c_sb[:, d : d + 1]
                    )
                else:
                    nc.scalar.activation(
                        tT_sb[:, d, :], tps[d], func=AF.Identity,
                        bias=c_sb[:, d : d + 1],
                    )


        # ---- Phase B: S chunks in PSUM + online softmax straight to E.
        def softmax_step(ps, i, j):
            if j == 0:
                nc.vector.reduce_max(
                    out=nmk[i][:, 0:1], in_=ps, axis=AX.X, negate=True
                )
            else:
                nc.vector.reduce_max(
                    out=tmx[i][:, j : j + 1], in_=ps, axis=AX.X, negate=True
                )
                nc.vector.tensor_tensor(
                    out=nmk[i][:, j : j + 1],
                    in0=nmk[i][:, j - 1 : j],
                    in1=tmx[i][:, j : j + 1],
                    op=ALU.min,
                )
            nc.scalar.activation(
                out=E_bf[i][:, j * 512 : (j + 1) * 512],
                in_=ps,
                func=AF.Exp,
                bias=nmk[i][:, j : j + 1],
                scale=1.0,
                accum_out=zpart[i][:, j : j + 1],
            )

        def finalize_pair(ia, ib):
            # c_k = exp(m_k - m_last), Z = sum z_k c_k, g = c_k/Z; then the
            # per-chunk diag(g) tiles, k-ordered round-robin across DVE/ACT
            # so both i-tiles' early-k diags finish first, in parallel.
            for i in (ia, ib):
                nc.scalar.activation(
                    out=ck[i],
                    in_=nmk[i],
                    func=AF.Exp,
                    bias=nmk[i][:, JC - 1 : JC],
                    scale=-1.0,
                )
            for i in (ia, ib):
                nc.vector.tensor_tensor(
                    out=gk[i], in0=zpart[i], in1=ck[i], op=ALU.mult
                )
            for i in (ia, ib):
                nc.vector.reduce_sum(out=zsum[i], in_=gk[i], axis=AX.X)
            for i in (ia, ib):
                nc.vector.reciprocal(rz[i], zsum[i])
            for i in (ia, ib):
                nc.vector.tensor_scalar_mul(gk[i], ck[i], rz[i])
            for k in range(JC):
                dve_i = ia if k % 2 == 0 else ib
                act_i = ib if k % 2 == 0 else ia
                nc.vector.tensor_scalar_mul(
                    diag[:, dve_i, k, :], ident, gk[dve_i][:, k : k + 1]
                )
                nc.scalar.activation(
                    diag[:, act_i, k, :],
                    ident,
                    func=AF.Copy,
                    scale=gk[act_i][:, k : k + 1],
                )

        bpend = []
        with ExitStack() as phb:
            spsum = phb.enter_context(tc.tile_pool(name="spsum", bufs=4, space="PSUM"))
            padpool = phb.enter_context(
                tc.tile_pool(name="padpool", bufs=1, space="PSUM")
            )
            def mm_group(ps, i, xtj):
                for d in range(EC):
                    nc.tensor.matmul(
                        ps,
                        tT_sb[:, d, i * PT : (i + 1) * PT],
                        xtj[:, d, :],
                        start=(d == 0),
                        stop=(d == EC - 1),
                    )

            for j in range(JC - 2):
                xtj = xts_sb if j == 0 else xtjs[j]
                if j > 0:
                    nc.sync.dma_start(xtj, xTb[j])
                for i in range(IT):
                    ps = spsum.tile([PT, 512], F32, tag="Sp", name="Sp")
                    mm_group(ps, i, xtj)
                    softmax_step(ps, i, j)
                if j == 0:
                    xspool_cm.__exit__(None, None, None)

            # Chunks 6-7 are processed i0/i1-first so their finalize chain
            # (which gates phase C's first transposes) completes ~2 PE groups
            # before B's compute ends. i3's last group goes to a dedicated
            # PSUM bank so its deferred exp gates nothing in phase C.
            nc.sync.dma_start(xtjs[JC - 2], xTb[JC - 2])
            nc.sync.dma_start(xtjs[JC - 1], xTb[JC - 1])
            xt6, xt7 = xtjs[JC - 2], xtjs[JC - 1]

            def subchain(i):
                # gk = ck * (1/Z); only the k0 diag is built inline (k1+ ride
                # the deferred stream, needed two jt-pairs later). i1's build
                # goes to ACT so it runs parallel to this DVE chain.
                nc.vector.tensor_tensor(
                    out=gk[i], in0=zpart[i], in1=ck[i], op=ALU.mult
                )
                nc.vector.reduce_sum(out=zsum[i], in_=gk[i], axis=AX.X)
                nc.vector.reciprocal(rz[i], zsum[i])
                nc.vector.tensor_scalar_mul(gk[i], ck[i], rz[i])
                if i == 0:
                    for k in (0, 1):
                        nc.vector.tensor_scalar_mul(
                            diag[:, 0, k, :], ident, gk[0][:, k : k + 1]
                        )

            for i in (0, 1):
                ps = spsum.tile([PT, 512], F32, tag="Sp", name="Sp")
                mm_group(ps, i, xt6)
                softmax_step(ps, i, JC - 2)
            ps70 = spsum.tile([PT, 512], F32, tag="Sp", name="Sp")
            mm_group(ps70, 0, xt7)
            ps71 = spsum.tile([PT, 512], F32, tag="Sp", name="Sp")
            mm_group(ps71, 1, xt7)
            softmax_step(ps70, 0, JC - 1)
            nc.scalar.activation(
                out=ck[0], in_=nmk[0], func=AF.Exp,
                bias=nmk[0][:, JC - 1 : JC], scale=-1.0,
            )
            softmax_step(ps71, 1, JC - 1)
            nc.scalar.activation(
                out=ck[1], in_=nmk[1], func=AF.Exp,
                bias=nmk[1][:, JC - 1 : JC], scale=-1.0,
            )
            subchain(0)
            subchain(1)
            # Banks 4-5 host phase C's first two transpose groups (hoisted
            # below), bank 6 stays an unwritten pad, and the final i3 group
            # lands on bank 7, which phase C never reallocates; its exp can
            # then be deferred into phase C's queues safely.
            # pad FIRST: tpsum's first rotating buf then lands on the
            # never-written pad bank, so phase C's g=2 transposes have zero
            # WAR; the pst banks' WARs (the hoisted et copies) clear before
            # the later bufs rotate onto them.
            padpool.tile([PT, 512], F32, tag="pad2", name="pad2")
            pst0 = padpool.tile([PT, 512], F32, tag="pst0", name="pst0")
            pst1 = padpool.tile([PT, 512], F32, tag="pst1", name="pst1")
            ps73 = padpool.tile([PT, 512], F32, tag="Spz", name="Spz")
            # Phase C's g=0,1 transpose groups run here, between B's last
            # two PE groups: their diag inputs (i0, chunks 0-1) were built
            # by subchain(0) ~4 PE groups ago, so the et copies drain on
            # DVE/ACT while B's last matmuls run and phase C's first oacc
            # matmuls start right at B's end.
            ets_pre = {}
            for g01 in (0, 1):
                pstX = pst0 if g01 == 0 else pst1
                for s in range(4):
                    jt = 4 * g01 + s
                    nc.tensor.matmul(
                        pstX[:, s * PT : (s + 1) * PT],
                        E_bf[0][:, jt * PT : (jt + 1) * PT],
                        diag[:, 0, g01, :],
                        start=True,
                        stop=True,
                        skip_group_check=True,
                    )
                if g01 % 2 == 0:
                    nc.vector.tensor_copy(etp[g01], pstX)
                else:
                    nc.scalar.activation(etp[g01], pstX, func=AF.Copy)
                ets_pre[g01] = etp[g01]
            # diag chunks 2-3 built here too: phase C's g=2,3 transposes
            # then start right at B's end instead of waiting for pops on
            # the congested post-B DVE queue.
            for k in (2, 3):
                nc.vector.tensor_scalar_mul(
                    diag[:, 0, k, :], ident, gk[0][:, k : k + 1]
                )
            for i in (2, 3):
                ps = spsum.tile([PT, 512], F32, tag="Sp", name="Sp")
                mm_group(ps, i, xt6)
                softmax_step(ps, i, JC - 2)
            ps72 = spsum.tile([PT, 512], F32, tag="Sp", name="Sp")
            mm_group(ps72, 2, xt7)
            softmax_step(ps72, 2, JC - 1)
            mm_group(ps73, 3, xt7)

            def red3min3():
                nc.vector.reduce_max(
                    out=tmx[3][:, JC - 1 : JC], in_=ps73, axis=AX.X, negate=True
                )
                nc.vector.tensor_tensor(
                    out=nmk[3][:, JC - 1 : JC], in0=nmk[3][:, JC - 2 : JC - 1],
                    in1=tmx[3][:, JC - 1 : JC], op=ALU.min,
                )

            def exp3():
                nc.scalar.activation(
                    out=E_bf[3][:, (JC - 1) * 512 : JC * 512],
                    in_=ps73, func=AF.Exp,
                    bias=nmk[3][:, JC - 1 : JC], scale=1.0,
                    accum_out=zpart[3][:, JC - 1 : JC],
                )

            def dgk01(k):
                # only pass 0 still uses diag tiles; i1's build is dead
                def emit():
                    nc.vector.tensor_scalar_mul(
                        diag[:, 0, k, :], ident, gk[0][:, k : k + 1]
                    )
                return emit

            def fin23a():
                for i in (2, 3):
                    nc.scalar.activation(
                        out=ck[i], in_=nmk[i], func=AF.Exp,
                        bias=nmk[i][:, JC - 1 : JC], scale=-1.0,
                    )
                for i in (2, 3):
                    nc.vector.tensor_tensor(
                        out=gk[i], in0=zpart[i], in1=ck[i], op=ALU.mult
                    )
                for i in (2, 3):
                    nc.vector.reduce_sum(out=zsum[i], in_=gk[i], axis=AX.X)
                for i in (2, 3):
                    nc.vector.reciprocal(rz[i], zsum[i])
                for i in (2, 3):
                    nc.vector.tensor_scalar_mul(gk[i], ck[i], rz[i])

            def dg23(k0, k1):
                def emit():
                    for k in range(k0, k1):
                        nc.vector.tensor_scalar_mul(
                            diag[:, 2, k, :], ident, gk[2][:, k : k + 1]
                        )
                        nc.scalar.activation(
                            diag[:, 3, k, :], ident, func=AF.Copy,
                            scale=gk[3][:, k : k + 1],
                        )
                return emit

            bpend.append(red3min3)
            bpend.append(exp3)
            for k in range(4, JC):
                bpend.append(dgk01(k))
            bpend.append(fin23a)

        # ---- Phase C: out = P @ x with x fully resident in SBUF.
        # xb reuses the phase-A weight pool's address range; its triggers sit
        # on the sync ring BEHIND the phase-B xtj stream: ring-FIFO order
        # guarantees zero bandwidth contention with the stream B chases, and
        # the ring is otherwise idle from ~57us, so all 8MB still land
        # before phase C's first pass needs its last key tiles.
        xbpool = big.enter_context(tc.tile_pool(name="xbpool", bufs=1))
        xb_sb = xbpool.tile([PT, JT, D], BF16)
        for qh in range(4):
            nc.sync.dma_start(
                xb_sb[:, qh * 8 : (qh + 1) * 8, :], xb[:, qh * 8 : (qh + 1) * 8, :]
            )
        etpool = big.enter_context(tc.tile_pool(name="etpool", bufs=4))
        ocopy = big.enter_context(tc.tile_pool(name="ocopy", bufs=2))
        # opsum allocated before tpsum: oacc lands on banks 0-3, whose last
        # phase-B exps clear 1.5-5us before B's end; tpsum gets the pad banks
        # (never written) plus ps73's bank (freed by the qv1-popped exp3).
        opsum = big.enter_context(tc.tile_pool(name="opsum", bufs=1, space="PSUM"))
        tpsum = big.enter_context(tc.tile_pool(name="tpsum", bufs=3, space="PSUM"))
        # One pass per i-tile so each pass's output drain (copy + 512KB DMA)
        # overlaps the next pass's ~15.6us of compute; only i3's drain is
        # exposed at the very end, split across both rings. Accumulator banks
        # alternate by pass parity so pass p's start=True matmuls never wait
        # on pass p-1's drain (only on p-2's, ~15us stale).
        oacc = {
            (par, dn): opsum.tile(
                [PT, 512], F32, tag=f"o{par}_{dn}", name=f"o{par}_{dn}"
            )
            for par in range(2)
            for dn in range(2)
        }

        def drain_item(i, par):
            # both dn halves into one [PT, D] tile (DVE + ACT in parallel),
            # then a single full-width DMA with 4KB rows, rings alternating.
            def emit():
                ot = ocopy.tile([PT, D], F32, tag="ot", name="ot")
                nc.vector.tensor_copy(ot[:, 0:512], oacc[(par, 0)])
                nc.scalar.activation(ot[:, 512:D], oacc[(par, 1)], func=AF.Copy)
                eng = nc.sync if i % 2 == 0 else nc.scalar
                eng.dma_start(out[i * PT : (i + 1) * PT, :], ot)

            return emit

        # Passes 1-3 skip the PE transposes entirely: their E tiles are
        # rescaled in place (E *= g_k, per-chunk per-row) on DVE/ACT one
        # pass ahead, then transposed by the DMA xbar (16x128 tiles,
        # ~3.6us per i-tile on the idle gpsimd ring) into etq buffers laid
        # out [key%128, key//128, q] - exactly the lhsT tiling the
        # accumulation matmuls need. Only pass 0, whose g_k are ready just
        # ~2 PE groups before B ends, keeps the diag-ride PE transposes.
        etqpool = big.enter_context(tc.tile_pool(name="etqpool", bufs=2))
        et_t = {
            i: etqpool.tile([PT, JT, PT], BF16, tag="etq", name=f"etq{i}")
            for i in (1, 2, 3)
        }

        def presc(i, k, eng):
            def emit():
                sl = slice(k * 512, (k + 1) * 512)
                if eng is nc.vector:
                    nc.vector.tensor_scalar_mul(
                        E_bf[i][:, sl], E_bf[i][:, sl], gk[i][:, k : k + 1]
                    )
                else:
                    nc.scalar.activation(
                        E_bf[i][:, sl], E_bf[i][:, sl], func=AF.Copy,
                        scale=gk[i][:, k : k + 1],
                    )
            return emit

        def tq(i):
            def emit():
                nc.sync.dma_start_transpose(et_t[i], E_bf[i])
            return emit

        pending2 = []
        for i in (1, 2, 3):
            for k in range(JC):
                pending2.append(
                    presc(i, k, nc.vector if k % 2 == 0 else nc.scalar)
                )
            pending2.append(tq(i))

        QV = JT // 4  # 8 groups of 4 jt; PE transposes exist for pass 0 only
        LOOKP = 3
        NG = IT * QV
        pending = bpend
        pending_drain = []
        ets = dict(ets_pre)  # groups 0-1 transposed + copied inside B's tail
        # One continuous software pipeline: pass 0's transpose groups (g<8)
        # interleave ahead of the oacc stream, which lags LOOKP behind and
        # crosses pass boundaries (parity-alternating accumulator banks).
        for g in range(2, NG + LOOKP):
            if g < NG:
                # pops lead each group: diag tiles for pass-0 chunk k=g are
                # emitted before the g transposes that read them; the
                # prescale/transpose stream for passes 1-3 drains alongside.
                if g >= 1:
                    for _ in range(2):
                        if pending:
                            pending.pop(0)()
                    for _ in range(3):
                        if pending2:
                            pending2.pop(0)()
                if g % QV == 3 and pending_drain:
                    pending_drain.pop(0)()
                if g < QV:
                    # pass-0 "transpose" = E_tile.T @ diag(g): softmax scale
                    # rides the mandatory transpose. Four jt per pst bank ->
                    # one PSUM->SBUF et copy per group (DVE/ACT alternating).
                    pst = tpsum.tile([PT, 512], F32, tag="tp", name="pst")
                    for s in range(4):
                        jt = 4 * g + s
                        nc.tensor.matmul(
                            pst[:, s * PT : (s + 1) * PT],
                            E_bf[0][:, jt * PT : (jt + 1) * PT],
                            diag[:, 0, g, :],
                            start=True,
                            stop=True,
                            skip_group_check=True,
                        )
                    et = etpool.tile([PT, 512], BF16, tag="et", name="et")
                    if g % 2 == 0:
                        nc.vector.tensor_copy(et, pst)
                    else:
                        nc.scalar.activation(et, pst, func=AF.Copy)
                    ets[g % 4] = et
            if g >= LOOKP:
                go = g - LOOKP
                p2, q = divmod(go, QV)
                par = p2 % 2
                for s in range(4):
                    jt = 4 * q + s
                    for dn in range(2):
                        lhsT = (
                            ets[go % 4][:, s * PT : (s + 1) * PT]
                            if p2 == 0
                            else et_t[p2][:, jt, :]
                        )
                        nc.tensor.matmul(
                            oacc[(par, dn)],
                            lhsT,
                            xb_sb[:, jt, dn * 512 : (dn + 1) * 512],
                            start=(jt == 0),
                            stop=(jt == JT - 1),
                        )
                if q == QV - 1:
                    if p2 < IT - 1:
                        pending_drain.append(drain_item(p2, par))
                    else:
                        # final drain: DVE+ACT copy the dn halves in
                        # parallel, then ROW-halves leave on both rings with
                        # full 4KB-row descriptors.
                        ot = ocopy.tile([PT, D], F32, tag="ot", name="ot")
                        nc.vector.tensor_copy(ot[:, 0:512], oacc[(par, 0)])
                        nc.scalar.activation(
                            ot[:, 512:D], oacc[(par, 1)], func=AF.Copy
                        )
                        nc.sync.dma_start(
                            out[p2 * PT : p2 * PT + 64, :], ot[0:64, :]
                        )
                        nc.scalar.dma_start(
                            out[p2 * PT + 64 : (p2 + 1) * PT, :], ot[64:PT, :]
                        )


def build():
    nc = bacc.Bacc(
        "TRN2",
        target_bir_lowering=False,
        debug=False,
        enable_asserts=False,
        num_devices=NCORES,
    )
    aps = {
        "xTb": nc.dram_tensor("xTb", [JC, PT, EC, 512], F16, kind="ExternalInput").ap(),
        "xTs": nc.dram_tensor("xTs", [D, R], F16, kind="ExternalInput").ap(),
        "mw": nc.dram_tensor("mw", [D, D], F16, kind="ExternalInput").ap(),
        "cw": nc.dram_tensor("cw", [PT, EC], F32, kind="ExternalInput").ap(),
        "xb": nc.dram_tensor("xb", [PT, JT, D], BF16, kind="ExternalInput").ap(),
        "out": nc.dram_tensor("out", [R, D], F32, kind="ExternalOutput").ap(),
    }
    with tile.TileContext(nc) as tc:
        _emit(nc, tc, aps)
    nc.compile()
    return nc


_NC_CACHE = None
LAST_RESULTS = None


def _get_nc():
    global _NC_CACHE
    if _NC_CACHE is None:
        _NC_CACHE = build()
    return _NC_CACHE


def make_in_maps(x, Wq, bq, Wk):
    x = np.ascontiguousarray(np.asarray(x, dtype=np.float32))
    xT = np.ascontiguousarray(x.T).astype(np.float16)
    # xTb[j, p, e, n] = xT[e*128 + p, j*512 + n]: per-(j,p) contiguous 16KB
    # blocks so the phase-B stream DMAs at full descriptor size.
    xTb = np.ascontiguousarray(
        xT.reshape(EC, PT, JC, 512).transpose(2, 1, 0, 3)
    )
    wk64 = np.asarray(Wk, dtype=np.float64)
    mw = np.ascontiguousarray(
        (np.asarray(Wq, dtype=np.float64).T @ wk64).astype(np.float16)
    )
    # cw[p, e] = c[e*128 + p]: per-partition bias column for the tT copies.
    cw = np.ascontiguousarray(
        (np.asarray(bq, dtype=np.float64) @ wk64)
        .astype(np.float32)
        .reshape(EC, PT)
        .T
    )
    xb = x.astype(ml_dtypes.bfloat16)
    in_maps = []
    for c in range(NCORES):
        # Each core processes key chunks in rotated order [c, c+1, ..]: its
        # own query slice xTs doubles as stream position 0 (already in SBUF
        # when phase B starts), so xTb and xb are rotated to match. The
        # rotation permutes softmax terms and P@x rows consistently; the
        # output rows (queries) are unaffected.
        in_maps.append(
            {
                "xTb": np.ascontiguousarray(
                    np.concatenate([xTb[c:], xTb[:c]], axis=0)
                ),
                "xTs": np.ascontiguousarray(xT[:, c * R : (c + 1) * R]),
                "mw": mw,
                "cw": cw,
                "xb": np.ascontiguousarray(
                    np.roll(xb, -512 * c, axis=0)
                    .reshape(JT, PT, D)
                    .transpose(1, 0, 2)
                ),
            }
        )
    return in_maps


def kernel(x, Wq, bq, Wk, bk):
    # bk only shifts each score row by a constant, which softmax cancels.
    del bk
    in_maps = make_in_maps(x, Wq, bq, Wk)
    nc = _get_nc()
    kwargs = {}
    if os.environ.get("K_TRACE_DIR"):
        import tempfile

        kwargs["tmpdir"] = tempfile.mkdtemp(dir=os.environ["K_TRACE_DIR"])
    res = run_bass_kernel_spmd(nc, in_maps, core_ids=list(range(NCORES)), **kwargs)
    global LAST_RESULTS
    LAST_RESULTS = res
    return np.concatenate(
        [np.asarray(res.results[c]["out"], dtype=np.float32) for c in range(NCORES)],
        axis=0,
    )



# revision 60
# speedup vs baseline: 1.0081x; 1.0081x over previous
"""Trainium2 Bass kernel for CLIP attention pooling.

Reference computation (N=4096, D=1024, fp32):
    q = x @ Wq.T + bq
    k = x @ Wk.T + bk
    attn = softmax(q @ k.T, axis=-1)
    out = attn @ x

Math notes:
  * scores = q @ k.T = q @ Wk @ x.T + (q.bk) 1^T. The (q.bk) term is
    constant along the softmax axis, so bk never needs to be computed.
  * q @ Wk = x @ (Wq.T @ Wk) + bq @ Wk: both projections fold into one
    matrix M = Wq.T @ Wk and a row c = bq @ Wk (host-precomputed).
  * Per core (512 query rows):
        tT = M^T . xs^T + c          [D, 512]   (transposed layout)
        S  = t . x^T                 [512, 4096]
        P  = softmax(S)              (online, running-max)
        out = P @ x                  [512, 1024]
  * The whole score path runs in fp16 (10-bit mantissa ~ fp32r's 11);
    scores accumulate in fp32 PSUM. E is bf16, out is fp32. fp8
    (DoubleRow 2x) was evaluated and fails the accuracy budget: e4m3
    quantization of P/x alone costs ~3e-2 L2 on the output.

Schedule (PE busy ~131us of ~149us exec; B and C run at the 216ns/
512-cycle matmul roofline with <100ns/group of slack):
  * phase A: M chunks stream on the sync HWDGE ring, xs chunks on the
    scalar ring (independent trigger FIFOs; the first-needed pieces
    lead both rings, xs' second e0 half rides sync so round e0 is not
    gated on the slow scalar trigger queue). M rides FULL 256KB chunk
    DMAs: trigger instructions cost ~650ns each on the issuing engine
    queue, so the stream is trigger-issue-limited and fewer, larger
    DMAs land it ~4us earlier. 42 tiny matmuls on a memset tile warm
    the PE p-state while the first chunks land - the DVFS ramp is
    activity-driven (half clock for ~5.5us) and the early DMA only
    sustains ~100-250GB/s, so real work cannot start before ~11.5us
    anyway and the warmups exactly cover ramp + DMA latency.
    e-outer over 8 PSUM banks for chunks e0-e4 (chasing the streams),
    then each bank finishes e5-e7 d-by-d so the 8 stops stagger
    ~0.65us apart; each bank's PSUM->SBUF copy (bias c folded in,
    full-width, engines alternating by d - R-range splits of one
    d-slice serialize across engines) chases its stop, and phase B's
    first group (which inherits ALL copies' deps via accumulation-
    group dep merging) starts ~0.8us after A's last matmul.
  * key-chunk rotation: each core processes key chunks in rotated order
    [c, c+1, ...]; its own query slice xTs IS rotated chunk 0 and is
    already in SBUF when phase B starts (zero DMA wait). xTb and xb are
    host-rotated to match; output rows are queries, so unaffected.
  * phase B: online softmax. Per (i, chunk): 8 accumulating matmuls
    into a PSUM bank, DVE keeps a running negated max straight out of
    PSUM, ACT applies exp(PSUM - runmax) into bf16 E with accum_out
    collecting per-chunk partial sums. No S buffer exists. Chunks 6-7
    run i0/i1-first so their finalize chain (c_k = exp(m_k - m_final),
    Z = sum z_k c_k, g_k = c_k/Z -> per-chunk diag(g) tiles) completes
    ~2 PE groups before B's last matmul; i3's final group sits on a
    dedicated PSUM bank (via pad tiles) so its exp can be deferred into
    phase C's queues without gating anything.
  * phase C: one pass per i-tile (out rows drain 3x earlier), emitted
    as a single continuous software pipeline; the oacc matmul groups
    lag LOOKP=3 behind and cross pass boundaries (accumulator banks
    alternate by pass parity), so no per-pass barrier exists.
  * E transposes: pass 0 transposes on the PE (E_tile.T @ diag(g)
    matmuls, 4 jt per pst bank + one PSUM->SBUF et copy per group -
    its g_k are only ready ~2 PE groups before B ends). Passes 1-3
    pay ZERO PE transpose cycles: their E tiles are rescaled in place
    (E *= g_k per chunk, DVE/ACT tensor-scalar, popped one pass
    ahead) and transposed by the DMA xbar (dma_start_transpose,
    16x128 hw tiles, ~3.6us per 1MB i-tile on the sync ring) into
    [key%128, key//128, q] etq buffers - exactly the lhsT tiling the
    accumulation matmuls need, ~3us before each pass consumes them.
    This cuts 24 of 32 transpose groups from the PE (~5us). Pass 0's
    first two transpose groups + et copies + the diag k2/k3 builds are
    hoisted into B right after subchain(1) - emitted any later they
    would head-block behind B's final softmax ops on the in-order
    DVE/ACT queues and stall C's entry by ~0.6us. Deferred phase-B
    finalize work pops ahead of the pass-0 transposes that read its
    diag tiles; drain pops wait for slot 3 of each pass.
  * x (bf16) is fully resident in SBUF: host-pretransposed to
    [128, 32, 1024], loaded in 4 quarters on the sync ring BEHIND the
    phase-B xtj stream (ring-FIFO order = zero bandwidth contention
    with the stream B chases; a third-ring/gpsimd variant measurably
    starved xtj and cost 2-5us of B). It aliases the released weight
    pools and still lands ~10us before phase C's first pass ends.
  * outputs: each 128-row block is copied from two PSUM banks into one
    [128, 1024] tile (DVE + ACT in parallel) and leaves via a single
    full-width DMA, rings alternating, hidden under the next pass; the
    final block leaves as ROW-halves on both rings with 4KB-row
    descriptors (2KB-row column-split DMAs only sustain ~100GB/s per
    queue).
"""

import os
from contextlib import ExitStack

import numpy as np
import ml_dtypes

import concourse.bass as bass
import concourse.mybir as mybir
import concourse.tile as tile
from concourse import bacc
from concourse.bass_utils import run_bass_kernel_spmd
from concourse.masks import make_identity

N, D = 4096, 1024
NCORES = 8
R = N // NCORES  # 512 query rows per core
PT = 128  # partition tile
EC = D // PT  # 8 contraction chunks of the model dim
IT = R // PT  # 4 query tiles per core
JC = N // 512  # 8 key chunks of 512
JT = N // PT  # 32 key tiles of 128

F32 = mybir.dt.float32
F32R = mybir.dt.float32r
F16 = mybir.dt.float16
BF16 = mybir.dt.bfloat16
AX = mybir.AxisListType
AF = mybir.ActivationFunctionType
ALU = mybir.AluOpType

PASSES = ((0, 1), (2,), (3,))


def _emit(nc: bass.Bass, tc: tile.TileContext, aps: dict):
    xTb, xTs, mw, cw, xb, out = (
        aps["xTb"], aps["xTs"], aps["mw"], aps["cw"],
        aps["xb"], aps["out"],
    )

    with ExitStack() as big:
        persist = big.enter_context(tc.tile_pool(name="persist", bufs=1))

        # warmup source: a plain memset tile (~100ns on gpsimd) so the PE
        # p-state ramp can start without waiting for the iota-based identity.
        wsrc = persist.tile([PT, PT], BF16)
        nc.gpsimd.memset(wsrc, 0.0)
        ident = persist.tile([PT, PT], BF16)
        make_identity(nc, ident)
        c_sb = persist.tile([PT, EC], F32)
        fence_sb = persist.tile([PT, 8], F16)

        tT_sb = persist.tile([PT, EC, R], F16)
        E_bf = [persist.tile([PT, N], BF16, name=f"E{i}") for i in range(IT)]
        nmk = [persist.tile([PT, JC], F32, name=f"nmk{i}") for i in range(IT)]
        tmx = [persist.tile([PT, JC], F32, name=f"tmx{i}") for i in range(IT)]
        zpart = [persist.tile([PT, JC], F32, name=f"zp{i}") for i in range(IT)]
        ck = [persist.tile([PT, JC], F32, name=f"ck{i}") for i in range(IT)]
        gk = [persist.tile([PT, JC], F32, name=f"gk{i}") for i in range(IT)]
        zsum = [persist.tile([PT, 1], F32, name=f"z{i}") for i in range(IT)]
        rz = [persist.tile([PT, 1], F32, name=f"rz{i}") for i in range(IT)]
        diag = persist.tile([PT, IT, JC, PT], BF16)
        # hoisted pass-0 et tiles (filled in B's tail; persist avoids any
        # pool-ordering constraints with xspool's mid-B close)
        etp = [persist.tile([PT, 512], BF16, name=f"etp{i}") for i in range(2)]

        # opened before wpool so its addresses never overlap the weights;
        # the early stream triggers can then issue during phase A.
        xtpool = big.enter_context(tc.tile_pool(name="xtpool", bufs=4))
        xtjs = {}
        for j in range(1, JC):
            xtjs[j] = xtpool.tile([PT, EC, 512], F16, tag="xtj", name="xtj")

        # xs in its own pool (opened after xtpool so pools unwind LIFO):
        # stream position 0 of phase B reads it directly - each core's query
        # slice IS its own key chunk - so it is only released after that;
        # the xb buffer then aliases it + wpool.
        xspool_cm = tc.tile_pool(name="xspool", bufs=1)
        xspool = xspool_cm.__enter__()
        xts_sb = xspool.tile([PT, EC, R], F16)

        # ---- Phase A: tT = M^T.xs^T + c  (transposed layout)
        with ExitStack() as pha:
            wpool = pha.enter_context(tc.tile_pool(name="wpool", bufs=1))
            apsum = pha.enter_context(tc.tile_pool(name="apsum", bufs=1, space="PSUM"))

            m_sb = wpool.tile([PT, EC, D], F16)

            m_r = mw.rearrange("(t p) d -> p t d", p=PT)
            xTs_r = xTs.rearrange("(t p) i -> p t i", p=PT)
            # M rides the sync HWDGE ring, xs + bias the scalar ring: the
            # trigger FIFOs are independent and the SDMA engines round-robin
            # between them. Chunk 0 of the phase-B stream is slotted in
            # before the last two M chunks: phase A's compute tail covers it.
            # First-needed pieces lead both rings: round e0 runs in R-halves,
            # so the very first matmuls need only m0a + the first 64KB of xs.
            # The e0h1 slice rides the sync ring (scalar's queue would issue
            # its trigger too late); c is tiny and not needed until A's tail,
            # so its trigger goes after the whole xs stream.
            nc.sync.dma_start(m_sb[:, 0, 0:PT], m_r[:, 0, 0:PT])
            nc.scalar.dma_start(xts_sb[:, 0, 0:256], xTs_r[:, 0, 0:256])
            nc.sync.dma_start(xts_sb[:, 0, 256:512], xTs_r[:, 0, 256:512])
            nc.sync.dma_start(m_sb[:, 0, PT:D], m_r[:, 0, PT:D])
            nc.scalar.dma_start(xts_sb[:, 1, :], xTs_r[:, 1, :])
            # full-chunk M DMAs: trigger instructions cost ~650ns each on
            # the sync engine queue, so 7 fewer triggers move the whole
            # M stream (and the xtj triggers behind it) ~4.5us earlier -
            # the stream is trigger-issue-limited, not bandwidth-limited.
            for e in range(1, EC):
                nc.sync.dma_start(m_sb[:, e, :], m_r[:, e, :])
            for e in range(2, EC):
                nc.scalar.dma_start(xts_sb[:, e, :], xTs_r[:, e, :])
            nc.scalar.dma_start(c_sb, cw)

            tps = [
                apsum.tile([PT, R], F32, tag=f"tp{d}", name=f"tp{d}")
                for d in range(EC)
            ]
            # PE warm-up only while the first DMA chunks are in flight
            # (~1.3us): the real phase-A matmuls then continue the p-state
            # ramp doing useful work at the reduced clock (the e-loop chases
            # the M/xs streams and stays stall-free even at half clock).
            # Results are clobbered by the first start=True matmul.
            for _ in range(42):
                nc.tensor.matmul(
                    tps[0][:, 0:PT], wsrc, wsrc, start=True, stop=True
                )
            # e-outer for chunks 0-4 (chasing the M/xs streams), then each
            # bank finishes its last 3 chunks d-by-d so the 8 stops stagger
            # ~0.65us apart instead of bunching in A's last round. Each
            # bank's PSUM->SBUF copy (bias c folded in) is issued right at
            # its stop, split into DVE+ACT halves running in parallel: the
            # copies fully chase the stops and B's first group (which
            # inherits ALL copies' deps via group-level merging) starts
            # ~0.4us after A's last matmul instead of ~1.3us.
            ESPLIT = 5
            for e in range(ESPLIT):
                for d in range(EC):
                    nc.tensor.matmul(
                        tps[d],
                        m_sb[:, e, d * PT : (d + 1) * PT],
                        xts_sb[:, e, :],
                        start=(e == 0),
                        stop=False,
                    )
            for d in range(EC):
                for e in range(ESPLIT, EC):
                    nc.tensor.matmul(
                        tps[d],
                        m_sb[:, e, d * PT : (d + 1) * PT],
                        xts_sb[:, e, :],
                        start=False,
                        stop=(e == EC - 1),
                    )
                # full-width copy, engines alternating by d: half-splits of
                # one d-slice get serialized DVE->ACT by the dep tracker.
                if d % 2 == 0:
                    nc.vector.tensor_scalar_add(
                        tT_sb[:, d, :], tps[d], c_sb[:, d : d + 1]
                    )
                else:
                    nc.scalar.activation(
                        tT_sb[:, d, :], tps[d], func=AF.Identity,
                        bias=c_sb[:, d : d + 1],
                    )


        # ---- Phase B: S chunks in PSUM + online softmax straight to E.
        def softmax_step(ps, i, j):
            if j == 0:
                nc.vector.reduce_max(
                    out=nmk[i][:, 0:1], in_=ps, axis=AX.X, negate=True
                )
            else:
                nc.vector.reduce_max(
                    out=tmx[i][:, j : j + 1], in_=ps, axis=AX.X, negate=True
                )
                nc.vector.tensor_tensor(
                    out=nmk[i][:, j : j + 1],
                    in0=nmk[i][:, j - 1 : j],
                    in1=tmx[i][:, j : j + 1],
                    op=ALU.min,
                )
            nc.scalar.activation(
                out=E_bf[i][:, j * 512 : (j + 1) * 512],
                in_=ps,
                func=AF.Exp,
                bias=nmk[i][:, j : j + 1],
                scale=1.0,
                accum_out=zpart[i][:, j : j + 1],
            )

        def finalize_pair(ia, ib):
            # c_k = exp(m_k - m_last), Z = sum z_k c_k, g = c_k/Z; then the
            # per-chunk diag(g) tiles, k-ordered round-robin across DVE/ACT
            # so both i-tiles' early-k diags finish first, in parallel.
            for i in (ia, ib):
                nc.scalar.activation(
                    out=ck[i],
                    in_=nmk[i],
                    func=AF.Exp,
                    bias=nmk[i][:, JC - 1 : JC],
                    scale=-1.0,
                )
            for i in (ia, ib):
                nc.vector.tensor_tensor(
                    out=gk[i], in0=zpart[i], in1=ck[i], op=ALU.mult
                )
            for i in (ia, ib):
                nc.vector.reduce_sum(out=zsum[i], in_=gk[i], axis=AX.X)
            for i in (ia, ib):
                nc.vector.reciprocal(rz[i], zsum[i])
            for i in (ia, ib):
                nc.vector.tensor_scalar_mul(gk[i], ck[i], rz[i])
            for k in range(JC):
                dve_i = ia if k % 2 == 0 else ib
                act_i = ib if k % 2 == 0 else ia
                nc.vector.tensor_scalar_mul(
                    diag[:, dve_i, k, :], ident, gk[dve_i][:, k : k + 1]
                )
                nc.scalar.activation(
                    diag[:, act_i, k, :],
                    ident,
                    func=AF.Copy,
                    scale=gk[act_i][:, k : k + 1],
                )

        bpend = []
        with ExitStack() as phb:
            spsum = phb.enter_context(tc.tile_pool(name="spsum", bufs=4, space="PSUM"))
            padpool = phb.enter_context(
                tc.tile_pool(name="padpool", bufs=1, space="PSUM")
            )
            def mm_group(ps, i, xtj):
                for d in range(EC):
                    nc.tensor.matmul(
                        ps,
                        tT_sb[:, d, i * PT : (i + 1) * PT],
                        xtj[:, d, :],
                        start=(d == 0),
                        stop=(d == EC - 1),
                    )

            for j in range(JC - 2):
                xtj = xts_sb if j == 0 else xtjs[j]
                if j > 0:
                    nc.sync.dma_start(xtj, xTb[j])
                for i in range(IT):
                    ps = spsum.tile([PT, 512], F32, tag="Sp", name="Sp")
                    mm_group(ps, i, xtj)
                    softmax_step(ps, i, j)
                if j == 0:
                    xspool_cm.__exit__(None, None, None)

            # Chunks 6-7 are processed i0/i1-first so their finalize chain
            # (which gates phase C's first transposes) completes ~2 PE groups
            # before B's compute ends. i3's last group goes to a dedicated
            # PSUM bank so its deferred exp gates nothing in phase C.
            nc.sync.dma_start(xtjs[JC - 2], xTb[JC - 2])
            nc.sync.dma_start(xtjs[JC - 1], xTb[JC - 1])
            xt6, xt7 = xtjs[JC - 2], xtjs[JC - 1]

            def subchain(i):
                # gk = ck * (1/Z); only the k0 diag is built inline (k1+ ride
                # the deferred stream, needed two jt-pairs later). i1's build
                # goes to ACT so it runs parallel to this DVE chain.
                nc.vector.tensor_tensor(
                    out=gk[i], in0=zpart[i], in1=ck[i], op=ALU.mult
                )
                nc.vector.reduce_sum(out=zsum[i], in_=gk[i], axis=AX.X)
                nc.vector.reciprocal(rz[i], zsum[i])
                nc.vector.tensor_scalar_mul(gk[i], ck[i], rz[i])
                if i == 0:
                    for k in (0, 1):
                        nc.vector.tensor_scalar_mul(
                            diag[:, 0, k, :], ident, gk[0][:, k : k + 1]
                        )

            for i in (0, 1):
                ps = spsum.tile([PT, 512], F32, tag="Sp", name="Sp")
                mm_group(ps, i, xt6)
                softmax_step(ps, i, JC - 2)
            ps70 = spsum.tile([PT, 512], F32, tag="Sp", name="Sp")
            mm_group(ps70, 0, xt7)
            ps71 = spsum.tile([PT, 512], F32, tag="Sp", name="Sp")
            mm_group(ps71, 1, xt7)
            softmax_step(ps70, 0, JC - 1)
            nc.scalar.activation(
                out=ck[0], in_=nmk[0], func=AF.Exp,
                bias=nmk[0][:, JC - 1 : JC], scale=-1.0,
            )
            softmax_step(ps71, 1, JC - 1)
            nc.scalar.activation(
                out=ck[1], in_=nmk[1], func=AF.Exp,
                bias=nmk[1][:, JC - 1 : JC], scale=-1.0,
            )
            subchain(0)
            subchain(1)
            # Banks 4-5 host phase C's first two transpose groups (hoisted
            # below), bank 6 stays an unwritten pad, and the final i3 group
            # lands on bank 7, which phase C never reallocates; its exp can
            # then be deferred into phase C's queues safely.
            # pad FIRST: tpsum's first rotating buf then lands on the
            # never-written pad bank, so phase C's g=2 transposes have zero
            # WAR; the pst banks' WARs (the hoisted et copies) clear before
            # the later bufs rotate onto them.
            padpool.tile([PT, 512], F32, tag="pad2", name="pad2")
            pst0 = padpool.tile([PT, 512], F32, tag="pst0", name="pst0")
            pst1 = padpool.tile([PT, 512], F32, tag="pst1", name="pst1")
            ps73 = padpool.tile([PT, 512], F32, tag="Spz", name="Spz")
            # Phase C's g=0,1 transpose groups run here, between B's last
            # two PE groups: their diag inputs (i0, chunks 0-1) were built
            # by subchain(0) ~4 PE groups ago, so the et copies drain on
            # DVE/ACT while B's last matmuls run and phase C's first oacc
            # matmuls start right at B's end.
            ets_pre = {}
            for g01 in (0, 1):
                pstX = pst0 if g01 == 0 else pst1
                for s in range(4):
                    jt = 4 * g01 + s
                    nc.tensor.matmul(
                        pstX[:, s * PT : (s + 1) * PT],
                        E_bf[0][:, jt * PT : (jt + 1) * PT],
                        diag[:, 0, g01, :],
                        start=True,
                        stop=True,
                        skip_group_check=True,
                    )
                if g01 % 2 == 0:
                    nc.vector.tensor_copy(etp[g01], pstX)
                else:
                    nc.scalar.activation(etp[g01], pstX, func=AF.Copy)
                ets_pre[g01] = etp[g01]
            # diag chunks 2-3 built here too: phase C's g=2,3 transposes
            # then start right at B's end instead of waiting for pops on
            # the congested post-B DVE queue.
            for k in (2, 3):
                nc.vector.tensor_scalar_mul(
                    diag[:, 0, k, :], ident, gk[0][:, k : k + 1]
                )
            for i in (2, 3):
                ps = spsum.tile([PT, 512], F32, tag="Sp", name="Sp")
                mm_group(ps, i, xt6)
                softmax_step(ps, i, JC - 2)
            ps72 = spsum.tile([PT, 512], F32, tag="Sp", name="Sp")
            mm_group(ps72, 2, xt7)
            softmax_step(ps72, 2, JC - 1)
            mm_group(ps73, 3, xt7)

            def red3min3():
                nc.vector.reduce_max(
                    out=tmx[3][:, JC - 1 : JC], in_=ps73, axis=AX.X, negate=True
                )
                nc.vector.tensor_tensor(
                    out=nmk[3][:, JC - 1 : JC], in0=nmk[3][:, JC - 2 : JC - 1],
                    in1=tmx[3][:, JC - 1 : JC], op=ALU.min,
                )

            def exp3():
                nc.scalar.activation(
                    out=E_bf[3][:, (JC - 1) * 512 : JC * 512],
                    in_=ps73, func=AF.Exp,
                    bias=nmk[3][:, JC - 1 : JC], scale=1.0,
                    accum_out=zpart[3][:, JC - 1 : JC],
                )

            def dgk01(k):
                # only pass 0 still uses diag tiles; i1's build is dead
                def emit():
                    nc.vector.tensor_scalar_mul(
                        diag[:, 0, k, :], ident, gk[0][:, k : k + 1]
                    )
                return emit

            def fin23a():
                for i in (2, 3):
                    nc.scalar.activation(
                        out=ck[i], in_=nmk[i], func=AF.Exp,
                        bias=nmk[i][:, JC - 1 : JC], scale=-1.0,
                    )
                for i in (2, 3):
                    nc.vector.tensor_tensor(
                        out=gk[i], in0=zpart[i], in1=ck[i], op=ALU.mult
                    )
                for i in (2, 3):
                    nc.vector.reduce_sum(out=zsum[i], in_=gk[i], axis=AX.X)
                for i in (2, 3):
                    nc.vector.reciprocal(rz[i], zsum[i])
                for i in (2, 3):
                    nc.vector.tensor_scalar_mul(gk[i], ck[i], rz[i])

            def dg23(k0, k1):
                def emit():
                    for k in range(k0, k1):
                        nc.vector.tensor_scalar_mul(
                            diag[:, 2, k, :], ident, gk[2][:, k : k + 1]
                        )
                        nc.scalar.activation(
                            diag[:, 3, k, :], ident, func=AF.Copy,
                            scale=gk[3][:, k : k + 1],
                        )
                return emit

            bpend.append(red3min3)
            bpend.append(exp3)
            for k in range(4, JC):
                bpend.append(dgk01(k))
            bpend.append(fin23a)

        # ---- Phase C: out = P @ x with x fully resident in SBUF.
        # xb reuses the phase-A weight pool's address range; its triggers sit
        # on the sync ring BEHIND the phase-B xtj stream: ring-FIFO order
        # guarantees zero bandwidth contention with the stream B chases, and
        # the ring is otherwise idle from ~57us, so all 8MB still land
        # before phase C's first pass needs its last key tiles.
        xbpool = big.enter_context(tc.tile_pool(name="xbpool", bufs=1))
        xb_sb = xbpool.tile([PT, JT, D], BF16)
        for qh in range(4):
            nc.sync.dma_start(
                xb_sb[:, qh * 8 : (qh + 1) * 8, :], xb[:, qh * 8 : (qh + 1) * 8, :]
            )
        etpool = big.enter_context(tc.tile_pool(name="etpool", bufs=4))
        ocopy = big.enter_context(tc.tile_pool(name="ocopy", bufs=2))
        # opsum allocated before tpsum: oacc lands on banks 0-3, whose last
        # phase-B exps clear 1.5-5us before B's end; tpsum gets the pad banks
        # (never written) plus ps73's bank (freed by the qv1-popped exp3).
        opsum = big.enter_context(tc.tile_pool(name="opsum", bufs=1, space="PSUM"))
        tpsum = big.enter_context(tc.tile_pool(name="tpsum", bufs=3, space="PSUM"))
        # One pass per i-tile so each pass's output drain (copy + 512KB DMA)
        # overlaps the next pass's ~15.6us of compute; only i3's drain is
        # exposed at the very end, split across both rings. Accumulator banks
        # alternate by pass parity so pass p's start=True matmuls never wait
        # on pass p-1's drain (only on p-2's, ~15us stale).
        # par1 allocated first: pass 0's accumulators (par0) then land on
        # the banks whose last phase-B users (ps70/ps71's exps) finished
        # ~9us before B ends, so C's first start=True matmuls have a
        # long-cleared WAR; par1's banks (last exps ~1.5us before B end)
        # aren't touched until pass 1, ~15us later.
        oacc = {
            (par, dn): opsum.tile(
                [PT, 512], F32, tag=f"o{par}_{dn}", name=f"o{par}_{dn}"
            )
            for par in (1, 0)
            for dn in range(2)
        }

        def drain_item(i, par):
            # both dn halves into one [PT, D] tile (DVE + ACT in parallel),
            # then a single full-width DMA with 4KB rows, rings alternating.
            def emit():
                ot = ocopy.tile([PT, D], F32, tag="ot", name="ot")
                nc.vector.tensor_copy(ot[:, 0:512], oacc[(par, 0)])
                nc.scalar.activation(ot[:, 512:D], oacc[(par, 1)], func=AF.Copy)
                eng = nc.sync if i % 2 == 0 else nc.scalar
                eng.dma_start(out[i * PT : (i + 1) * PT, :], ot)

            return emit

        # Passes 1-3 skip the PE transposes entirely: their E tiles are
        # rescaled in place (E *= g_k, per-chunk per-row) on DVE/ACT one
        # pass ahead, then transposed by the DMA xbar (16x128 tiles,
        # ~3.6us per i-tile on the idle gpsimd ring) into etq buffers laid
        # out [key%128, key//128, q] - exactly the lhsT tiling the
        # accumulation matmuls need. Only pass 0, whose g_k are ready just
        # ~2 PE groups before B ends, keeps the diag-ride PE transposes.
        etqpool = big.enter_context(tc.tile_pool(name="etqpool", bufs=2))
        et_t = {
            i: etqpool.tile([PT, JT, PT], BF16, tag="etq", name=f"etq{i}")
            for i in (1, 2, 3)
        }

        def presc(i, k, eng):
            def emit():
                sl = slice(k * 512, (k + 1) * 512)
                if eng is nc.vector:
                    nc.vector.tensor_scalar_mul(
                        E_bf[i][:, sl], E_bf[i][:, sl], gk[i][:, k : k + 1]
                    )
                else:
                    nc.scalar.activation(
                        E_bf[i][:, sl], E_bf[i][:, sl], func=AF.Copy,
                        scale=gk[i][:, k : k + 1],
                    )
            return emit

        def tq(i):
            def emit():
                nc.sync.dma_start_transpose(et_t[i], E_bf[i])
            return emit

        pending2 = []
        for i in (1, 2, 3):
            for k in range(JC):
                pending2.append(
                    presc(i, k, nc.vector if k % 2 == 0 else nc.scalar)
                )
            pending2.append(tq(i))

        QV = JT // 4  # 8 groups of 4 jt; PE transposes exist for pass 0 only
        LOOKP = 3
        NG = IT * QV
        pending = bpend
        pending_drain = []
        ets = dict(ets_pre)  # groups 0-1 transposed + copied inside B's tail
        # One continuous software pipeline: pass 0's transpose groups (g<8)
        # interleave ahead of the oacc stream, which lags LOOKP behind and
        # crosses pass boundaries (parity-alternating accumulator banks).
        for g in range(2, NG + LOOKP):
            if g < NG:
                # pops lead each group: diag tiles for pass-0 chunk k=g are
                # emitted before the g transposes that read them; the
                # prescale/transpose stream for passes 1-3 drains alongside.
                if g >= 1:
                    for _ in range(2):
                        if pending:
                            pending.pop(0)()
                    for _ in range(3):
                        if pending2:
                            pending2.pop(0)()
                if g % QV == 3 and pending_drain:
                    pending_drain.pop(0)()
                if g < QV:
                    # pass-0 "transpose" = E_tile.T @ diag(g): softmax scale
                    # rides the mandatory transpose. Four jt per pst bank ->
                    # one PSUM->SBUF et copy per group (DVE/ACT alternating).
                    pst = tpsum.tile([PT, 512], F32, tag="tp", name="pst")
                    for s in range(4):
                        jt = 4 * g + s
                        nc.tensor.matmul(
                            pst[:, s * PT : (s + 1) * PT],
                            E_bf[0][:, jt * PT : (jt + 1) * PT],
                            diag[:, 0, g, :],
                            start=True,
                            stop=True,
                            skip_group_check=True,
                        )
                    et = etpool.tile([PT, 512], BF16, tag="et", name="et")
                    if g % 2 == 0:
                        nc.vector.tensor_copy(et, pst)
                    else:
                        nc.scalar.activation(et, pst, func=AF.Copy)
                    ets[g % 4] = et
            if g >= LOOKP:
                go = g - LOOKP
                p2, q = divmod(go, QV)
                par = p2 % 2
                for s in range(4):
                    jt = 4 * q + s
                    for dn in range(2):
                        lhsT = (
                            ets[go % 4][:, s * PT : (s + 1) * PT]
                            if p2 == 0
                            else et_t[p2][:, jt, :]
                        )
                        nc.tensor.matmul(
                            oacc[(par, dn)],
                            lhsT,
                            xb_sb[:, jt, dn * 512 : (dn + 1) * 512],
                            start=(jt == 0),
                            stop=(jt == JT - 1),
                        )
                if q == QV - 1:
                    if p2 < IT - 1:
                        pending_drain.append(drain_item(p2, par))
                    else:
                        # final drain: DVE+ACT copy the dn halves in
                        # parallel, then ROW-halves leave on both rings with
                        # full 4KB-row descriptors.
                        ot = ocopy.tile([PT, D], F32, tag="ot", name="ot")
                        nc.vector.tensor_copy(ot[:, 0:512], oacc[(par, 0)])
                        nc.scalar.activation(
                            ot[:, 512:D], oacc[(par, 1)], func=AF.Copy
                        )
                        nc.sync.dma_start(
                            out[p2 * PT : p2 * PT + 64, :], ot[0:64, :]
                        )
                        nc.scalar.dma_start(
                            out[p2 * PT + 64 : (p2 + 1) * PT, :], ot[64:PT, :]
                        )


def build():
    nc = bacc.Bacc(
        "TRN2",
        target_bir_lowering=False,
        debug=False,
        enable_asserts=False,
        num_devices=NCORES,
    )
    aps = {
        "xTb": nc.dram_tensor("xTb", [JC, PT, EC, 512], F16, kind="ExternalInput").ap(),
        "xTs": nc.dram_tensor("xTs", [D, R], F16, kind="ExternalInput").ap(),
        "mw": nc.dram_tensor("mw", [D, D], F16, kind="ExternalInput").ap(),
        "cw": nc.dram_tensor("cw", [PT, EC], F32, kind="ExternalInput").ap(),
        "xb": nc.dram_tensor("xb", [PT, JT, D], BF16, kind="ExternalInput").ap(),
        "out": nc.dram_tensor("out", [R, D], F32, kind="ExternalOutput").ap(),
    }
    with tile.TileContext(nc) as tc:
        _emit(nc, tc, aps)
    nc.compile()
    return nc


_NC_CACHE = None
LAST_RESULTS = None


def _get_nc():
    global _NC_CACHE
    if _NC_CACHE is None:
        _NC_CACHE = build()
    return _NC_CACHE


def make_in_maps(x, Wq, bq, Wk):
    x = np.ascontiguousarray(np.asarray(x, dtype=np.float32))
    xT = np.ascontiguousarray(x.T).astype(np.float16)
    # xTb[j, p, e, n] = xT[e*128 + p, j*512 + n]: per-(j,p) contiguous 16KB
    # blocks so the phase-B stream DMAs at full descriptor size.
    xTb = np.ascontiguousarray(
        xT.reshape(EC, PT, JC, 512).transpose(2, 1, 0, 3)
    )
    wk64 = np.asarray(Wk, dtype=np.float64)
    mw = np.ascontiguousarray(
        (np.asarray(Wq, dtype=np.float64).T @ wk64).astype(np.float16)
    )
    # cw[p, e] = c[e*128 + p]: per-partition bias column for the tT copies.
    cw = np.ascontiguousarray(
        (np.asarray(bq, dtype=np.float64) @ wk64)
        .astype(np.float32)
        .reshape(EC, PT)
        .T
    )
    xb = x.astype(ml_dtypes.bfloat16)
    in_maps = []
    for c in range(NCORES):
        # Each core processes key chunks in rotated order [c, c+1, ..]: its
        # own query slice xTs doubles as stream position 0 (already in SBUF
        # when phase B starts), so xTb and xb are rotated to match. The
        # rotation permutes softmax terms and P@x rows consistently; the
        # output rows (queries) are unaffected.
        in_maps.append(
            {
                "xTb": np.ascontiguousarray(
                    np.concatenate([xTb[c:], xTb[:c]], axis=0)
                ),
                "xTs": np.ascontiguousarray(xT[:, c * R : (c + 1) * R]),
                "mw": mw,
                "cw": cw,
                "xb": np.ascontiguousarray(
                    np.roll(xb, -512 * c, axis=0)
                    .reshape(JT, PT, D)
                    .transpose(1, 0, 2)
                ),
            }
        )
    return in_maps


def kernel(x, Wq, bq, Wk, bk):
    # bk only shifts each score row by a constant, which softmax cancels.
    del bk
    in_maps = make_in_maps(x, Wq, bq, Wk)
    nc = _get_nc()
    kwargs = {}
    if os.environ.get("K_TRACE_DIR"):
        import tempfile

        kwargs["tmpdir"] = tempfile.mkdtemp(dir=os.environ["K_TRACE_DIR"])
    res = run_bass_kernel_spmd(nc, in_maps, core_ids=list(range(NCORES)), **kwargs)
    global LAST_RESULTS
    LAST_RESULTS = res
    return np.concatenate(
        [np.asarray(res.results[c]["out"], dtype=np.float32) for c in range(NCORES)],
        axis=0,
    )

